# revision 5
# baseline (speedup 1.0000x reference)
"""Trainium2 Bass kernel: GNN heat-conduction message passing.

Contract: kernel(**inputs) takes FULL numpy inputs (T, cp, L, conductivity,
A, time_step, src, dst) and returns the FULL [N] output
(heat_received - heat_sent), computed on 8 NeuronCores.

Strategy (edge-parallel, per sharding hint):
  - Shard the 6.4M edges across 8 cores (800k each, padded with zero-effect
    edges); replicate the small [N] node arrays T/cp to every core as an
    interleaved (T, cp) table in DRAM.
  - Per core: stream edge tiles of 128x128; gather (T, cp) pairs for src and
    dst endpoints via indirect DMA (embedding-gather pattern, 128 edges per
    call); per-edge math on DVE/ACT; segment-sums via per-column one-hot
    matmuls accumulated in two persistent PSUM banks laid out [r=128, q=784]
    where node n = 128*q + r.
  - Host unshard: reorder the [128, 784] partials to [N] and sum the 8 cores'
    (received - sent) partials.
"""

import numpy as np

import concourse.bacc as bacc
import concourse.bass as bass
import concourse.mybir as mybir
import concourse.tile as tile
from concourse.bass_utils import run_bass_kernel_spmd

N_NODES = 100_000
N_EDGES = 6_400_000
N_CORES = 8
P = 128
QDIM = 784  # node n = 128*q + r, q < ceil(100000/128)=782, padded to 784

F32 = mybir.dt.float32
F16 = mybir.dt.float16
I32 = mybir.dt.int32

_CACHE = {}


def build_kernel(cols, iters, n_cores):
    """Build the Bass module. Edge capacity per core = 128*cols*iters."""
    tile_e = P * cols
    epc = tile_e * iters

    nc = bacc.Bacc("TRN2", target_bir_lowering=False, debug=False,
                   enable_asserts=False, num_devices=n_cores)

    src_d = nc.dram_tensor("src", [epc], I32, kind="ExternalInput")
    dst_d = nc.dram_tensor("dst", [epc], I32, kind="ExternalInput")
    l_d = nc.dram_tensor("lw", [epc], F32, kind="ExternalInput")
    k_d = nc.dram_tensor("kw", [epc], F32, kind="ExternalInput")
    a_d = nc.dram_tensor("aw", [epc], F32, kind="ExternalInput")
    tab_d = nc.dram_tensor("tab", [N_NODES, 2], F32, kind="ExternalInput")
    ts_d = nc.dram_tensor("ts", [1, 1], F32, kind="ExternalInput")
    sent_d = nc.dram_tensor("sentp", [P, QDIM], F32, kind="ExternalOutput")
    recv_d = nc.dram_tensor("recvp", [P, QDIM], F32, kind="ExternalOutput")

    with tile.TileContext(nc) as tc:
        with (
            tc.tile_pool(name="const", bufs=1) as cpool,
            tc.tile_pool(name="grid", bufs=2) as gpool,
            tc.tile_pool(name="col", bufs=3) as colpool,
            tc.tile_pool(name="psum", bufs=1, space="PSUM") as ppool,
        ):
            # ---- constants ----
            iota128_i = cpool.tile([P, P], I32)
            nc.gpsimd.iota(iota128_i[:], pattern=[[1, P]], base=0,
                           channel_multiplier=0)
            iota128 = cpool.tile([P, P], F32)
            nc.vector.tensor_copy(out=iota128[:], in_=iota128_i[:])
            iotaq_i = cpool.tile([P, QDIM], I32)
            nc.gpsimd.iota(iotaq_i[:], pattern=[[1, QDIM]], base=0,
                           channel_multiplier=0)
            iotaq = cpool.tile([P, QDIM], F32)
            nc.vector.tensor_copy(out=iotaq[:], in_=iotaq_i[:])

            ts_sb = cpool.tile([1, 1], F32)
            nc.sync.dma_start(out=ts_sb[:], in_=ts_d[:])
            ts_b = cpool.tile([P, 1], F32)
            nc.gpsimd.partition_broadcast(ts_b[:], ts_sb[:])

            zeroq = cpool.tile([P, QDIM], F32)
            nc.vector.memset(zeroq[:], 0.0)
            zh = cpool.tile([P, P], F16)
            nc.vector.memset(zh[:], 0.0)
            zq = cpool.tile([P, QDIM], F16)
            nc.vector.memset(zq[:], 0.0)

            psum_s = ppool.tile([P, QDIM], F32)
            psum_r = ppool.tile([P, QDIM], F32)
            acc_s = cpool.tile([P, QDIM], F32)
            acc_r = cpool.tile([P, QDIM], F32)
            nc.vector.tensor_copy(out=acc_s[:], in_=zeroq[:])
            nc.vector.tensor_copy(out=acc_r[:], in_=zeroq[:])

            def body(it):
                base = it * tile_e

                def load(dram, dt, tag):
                    t = gpool.tile([P, cols], dt, tag=tag)
                    ap = dram[bass.ds(base, tile_e)].rearrange(
                        "(p c) -> p c", c=cols)
                    nc.sync.dma_start(out=t[:], in_=ap)
                    return t

                src_g = load(src_d, I32, "src_g")
                dst_g = load(dst_d, I32, "dst_g")
                l_g = load(l_d, F32, "l_g")
                k_g = load(k_d, F32, "k_g")
                a_g = load(a_d, F32, "a_g")

                # index fields -> fp16 grids
                def rq(g, sfx):
                    ri = gpool.tile([P, cols], I32, tag="ri" + sfx)
                    qi = gpool.tile([P, cols], I32, tag="qi" + sfx)
                    nc.vector.tensor_scalar(out=ri[:], in0=g[:], scalar1=127,
                                            scalar2=None,
                                            op0=mybir.AluOpType.bitwise_and)
                    nc.vector.tensor_scalar(
                        out=qi[:], in0=g[:], scalar1=7, scalar2=None,
                        op0=mybir.AluOpType.logical_shift_right)
                    rf = gpool.tile([P, cols], F32, tag="rf" + sfx)
                    qf = gpool.tile([P, cols], F32, tag="qf" + sfx)
                    nc.vector.tensor_copy(out=rf[:], in_=ri[:])
                    nc.vector.tensor_copy(out=qf[:], in_=qi[:])
                    return rf, qf

                rs_f, qs_f = rq(src_g, "s")
                rd_f, qd_f = rq(dst_g, "d")

                # gathers: (T, cp) pairs for src and dst; one column per call
                g_s = gpool.tile([P, cols, 2], F32, tag="g_s")
                g_d = gpool.tile([P, cols, 2], F32, tag="g_d")
                for c in range(cols):
                    nc.gpsimd.indirect_dma_start(
                        out=g_s[:, c, :], out_offset=None, in_=tab_d[:],
                        in_offset=bass.IndirectOffsetOnAxis(
                            ap=src_g[:, c:c + 1], axis=0))
                    nc.gpsimd.indirect_dma_start(
                        out=g_d[:, c, :], out_offset=None, in_=tab_d[:],
                        in_offset=bass.IndirectOffsetOnAxis(
                            ap=dst_g[:, c:c + 1], axis=0))

                t_s, cp_s = g_s[:, :, 0], g_s[:, :, 1]
                t_d, cp_d = g_d[:, :, 0], g_d[:, :, 1]

                def tt(a, b, op, tag, dt=F32):
                    o = gpool.tile([P, cols], dt, tag=tag)
                    nc.vector.tensor_tensor(out=o[:], in0=a, in1=b, op=op)
                    return o

                OP = mybir.AluOpType
                delta = tt(t_s, t_d, OP.subtract, "delta")
                nc.vector.tensor_scalar_max(out=delta[:], in0=delta[:],
                                            scalar1=0.0)
                rl = gpool.tile([P, cols], F32, tag="rl")
                nc.vector.reciprocal(out=rl[:], in_=l_g[:])
                grad = tt(delta[:], rl[:], OP.mult, "grad")
                gk = tt(grad[:], k_g[:], OP.mult, "gk")
                lng = gpool.tile([P, cols], F32, tag="lng")
                nc.scalar.activation(out=lng[:], in_=gk[:],
                                     func=mybir.ActivationFunctionType.Ln)
                hfd = gpool.tile([P, cols], F32, tag="hfd")
                nc.scalar.activation(out=hfd[:], in_=lng[:],
                                     func=mybir.ActivationFunctionType.Exp,
                                     scale=1.0 / 3.0)
                ec = tt(hfd[:], a_g[:], OP.mult, "ec")
                nc.vector.tensor_scalar(out=ec[:], in0=ec[:],
                                        scalar1=ts_b[:, 0:1], scalar2=None,
                                        op0=OP.mult)
                num = tt(cp_s, cp_d, OP.mult, "num")
                den = tt(cp_s, cp_d, OP.add, "den")
                nc.vector.reciprocal(out=den[:], in_=den[:])
                cpc = tt(num[:], den[:], OP.mult, "cpc")
                maxe = tt(delta[:], cpc[:], OP.mult, "maxe")
                e32 = tt(ec[:], maxe[:], OP.min, "e32")

                # scatter: per column one-hot matmuls into psum
                for c in range(cols):
                    hot_s = colpool.tile([P, P], F16, tag="hot_s")
                    nc.vector.tensor_scalar(
                        out=hot_s[:], in0=iota128[:],
                        scalar1=rs_f[:, c:c + 1], scalar2=e32[:, c:c + 1],
                        op0=OP.is_equal, op1=OP.mult)
                    qhot_s = colpool.tile([P, QDIM], F16, tag="qhot_s")
                    nc.vector.tensor_scalar(
                        out=qhot_s[:], in0=iotaq[:],
                        scalar1=qs_f[:, c:c + 1], scalar2=None,
                        op0=OP.is_equal)
                    hot_r = colpool.tile([P, P], F16, tag="hot_r")
                    nc.vector.tensor_scalar(
                        out=hot_r[:], in0=iota128[:],
                        scalar1=rd_f[:, c:c + 1], scalar2=e32[:, c:c + 1],
                        op0=OP.is_equal, op1=OP.mult)
                    qhot_r = colpool.tile([P, QDIM], F16, tag="qhot_r")
                    nc.vector.tensor_scalar(
                        out=qhot_r[:], in0=iotaq[:],
                        scalar1=qd_f[:, c:c + 1], scalar2=None,
                        op0=OP.is_equal)
                    first = c == 0
                    last = c == cols - 1
                    for lo, hi in ((0, 512), (512, QDIM)):
                        nc.tensor.matmul(out=psum_s[:, lo:hi], lhsT=hot_s[:],
                                         rhs=qhot_s[:, lo:hi], start=first,
                                         stop=last)
                        nc.tensor.matmul(out=psum_r[:, lo:hi], lhsT=hot_r[:],
                                         rhs=qhot_r[:, lo:hi], start=first,
                                         stop=last)

                # fold this tile's psum into the SBUF accumulators
                nc.vector.tensor_add(out=acc_s[:], in0=acc_s[:],
                                     in1=psum_s[:])
                nc.vector.tensor_add(out=acc_r[:], in0=acc_r[:],
                                     in1=psum_r[:])

            if iters > 1:
                with tc.For_i(0, iters, 1) as it:
                    body(it)
            else:
                body(0)

            nc.sync.dma_start(out=sent_d[:], in_=acc_s[:])
            nc.sync.dma_start(out=recv_d[:], in_=acc_r[:])

    nc.compile()
    return nc


def get_kernel(cols, iters, n_cores):
    key = (cols, iters, n_cores)
    if key not in _CACHE:
        _CACHE[key] = build_kernel(*key)
    return _CACHE[key]


def kernel(T, cp, L, conductivity, A, time_step, src, dst):
    T = np.asarray(T, np.float32)
    cp = np.asarray(cp, np.float32)
    L = np.asarray(L, np.float32)
    conductivity = np.asarray(conductivity, np.float32)
    A = np.asarray(A, np.float32)
    time_step = np.asarray(time_step, np.float32)
    src = np.asarray(src, np.int32)
    dst = np.asarray(dst, np.int32)

    n = T.shape[0]
    e = src.shape[0]

    cols = 128
    tile_e = P * cols
    epc_raw = (e + N_CORES - 1) // N_CORES
    iters = (epc_raw + tile_e - 1) // tile_e
    epc = iters * tile_e

    nc = get_kernel(cols, iters, N_CORES)

    tab = np.empty((N_NODES, 2), np.float32)
    tab[:n, 0] = T
    tab[:n, 1] = cp
    tab[n:] = 0.0

    in_maps = []
    for c in range(N_CORES):
        lo, hi = c * epc_raw, min((c + 1) * epc_raw, e)
        cnt = hi - lo

        def pad(x, fill, dt):
            out = np.full(epc, fill, dt)
            out[:cnt] = x[lo:hi]
            return out

        in_maps.append({
            "src": pad(src, 0, np.int32),
            "dst": pad(dst, 0, np.int32),
            "lw": pad(L, 1.0, np.float32),
            "kw": pad(conductivity, 0.0, np.float32),
            "aw": pad(A, 0.0, np.float32),
            "tab": tab,
            "ts": time_step.reshape(1, 1),
        })

    res = run_bass_kernel_spmd(nc, in_maps, core_ids=list(range(N_CORES)))

    acc = np.zeros((P, QDIM), np.float64)
    for c in range(N_CORES):
        acc += res.results[c]["recvp"].astype(np.float64)
        acc -= res.results[c]["sentp"].astype(np.float64)
    # node n = 128*q + r lives at [r, q]
    full = acc.T.ravel()[:n]
    return full.astype(np.float32)


# revision 9
# speedup vs baseline: 84.1896x; 84.1896x over previous
"""Trainium2 Bass kernel: GNN heat-conduction message passing.

Contract: kernel(**inputs) takes FULL numpy inputs (T, cp, L, conductivity,
A, time_step, src, dst) and returns the FULL [N] output
(heat_received - heat_sent), computed on 8 NeuronCores.

Strategy (edge-parallel, per sharding hint):
  - Shard the 6.4M edges across 8 cores (800k each, padded with zero-effect
    edges); replicate the small [N] node arrays T/cp to every core as an
    interleaved (T, cp) table in DRAM.
  - Per core: stream edge tiles of 128x128; gather (T, cp) pairs for src and
    dst endpoints via indirect DMA (embedding-gather pattern, 128 edges per
    call); per-edge math on DVE/ACT; segment-sums via per-column one-hot
    matmuls accumulated in two persistent PSUM banks laid out [r=128, q=784]
    where node n = 128*q + r.
  - Host unshard: reorder the [128, 784] partials to [N] and sum the 8 cores'
    (received - sent) partials.
"""

import numpy as np

import concourse.bacc as bacc
import concourse.bass as bass
import concourse.mybir as mybir
import concourse.tile as tile
from concourse.bass_utils import run_bass_kernel_spmd

N_NODES = 100_000
N_EDGES = 6_400_000
N_CORES = 8
P = 128
QDIM = 784  # node n = 128*q + r, q < ceil(100000/128)=782, padded to 784

F32 = mybir.dt.float32
F16 = mybir.dt.float16
I32 = mybir.dt.int32

_CACHE = {}


def build_kernel(cols, iters, n_cores):
    """Build the Bass module. Edge capacity per core = 128*cols*iters."""
    tile_e = P * cols
    epc = tile_e * iters

    nc = bacc.Bacc("TRN2", target_bir_lowering=False, debug=False,
                   enable_asserts=False, num_devices=n_cores)

    src_d = nc.dram_tensor("src", [epc], I32, kind="ExternalInput")
    dst_d = nc.dram_tensor("dst", [epc], I32, kind="ExternalInput")
    l_d = nc.dram_tensor("lw", [epc], F32, kind="ExternalInput")
    k_d = nc.dram_tensor("kw", [epc], F32, kind="ExternalInput")
    a_d = nc.dram_tensor("aw", [epc], F32, kind="ExternalInput")
    tab_d = nc.dram_tensor("tab", [N_NODES, 2], F32, kind="ExternalInput")
    ts_d = nc.dram_tensor("ts", [1, 1], F32, kind="ExternalInput")
    sent_d = nc.dram_tensor("sentp", [P, QDIM], F32, kind="ExternalOutput")
    recv_d = nc.dram_tensor("recvp", [P, QDIM], F32, kind="ExternalOutput")

    with tile.TileContext(nc) as tc:
        with (
            tc.tile_pool(name="const", bufs=1) as cpool,
            tc.tile_pool(name="grid", bufs=3) as gpool,
            tc.tile_pool(name="col", bufs=6) as colpool,
            tc.tile_pool(name="psum", bufs=1, space="PSUM") as ppool,
        ):
            # ---- constants ----
            iota128_i = cpool.tile([P, P], I32)
            nc.gpsimd.iota(iota128_i[:], pattern=[[1, P]], base=0,
                           channel_multiplier=0)
            iota128 = cpool.tile([P, P], F16)
            nc.vector.tensor_copy(out=iota128[:], in_=iota128_i[:])
            iotaq_i = cpool.tile([P, QDIM], I32)
            nc.gpsimd.iota(iotaq_i[:], pattern=[[1, QDIM]], base=0,
                           channel_multiplier=0)
            iotaq = cpool.tile([P, QDIM], F16)
            nc.vector.tensor_copy(out=iotaq[:], in_=iotaq_i[:])

            ts_sb = cpool.tile([1, 1], F32)
            nc.sync.dma_start(out=ts_sb[:], in_=ts_d[:])
            ts_b = cpool.tile([P, 1], F32)
            nc.gpsimd.partition_broadcast(ts_b[:], ts_sb[:])

            zeroq = cpool.tile([P, QDIM], F32)
            nc.vector.memset(zeroq[:], 0.0)
            zh = cpool.tile([P, P], F16)
            nc.vector.memset(zh[:], 0.0)
            zq = cpool.tile([P, QDIM], F16)
            nc.vector.memset(zq[:], 0.0)

            psum_s = ppool.tile([P, QDIM], F32)
            psum_r = ppool.tile([P, QDIM], F32)
            acc_s = cpool.tile([P, QDIM], F32)
            acc_r = cpool.tile([P, QDIM], F32)
            nc.vector.tensor_copy(out=acc_s[:], in_=zeroq[:])
            nc.vector.tensor_copy(out=acc_r[:], in_=zeroq[:])

            def body(it):
                base = it * tile_e

                def load(dram, dt, tag):
                    t = gpool.tile([P, cols], dt, tag=tag)
                    ap = dram[bass.ds(base, tile_e)].rearrange(
                        "(p c) -> p c", c=cols)
                    nc.sync.dma_start(out=t[:], in_=ap)
                    return t

                src_g = load(src_d, I32, "src_g")
                dst_g = load(dst_d, I32, "dst_g")
                l_g = load(l_d, F32, "l_g")
                k_g = load(k_d, F32, "k_g")
                a_g = load(a_d, F32, "a_g")

                # index fields -> fp16 grids
                def rq(g, sfx):
                    ri = gpool.tile([P, cols], I32, tag="ri" + sfx)
                    qi = gpool.tile([P, cols], I32, tag="qi" + sfx)
                    nc.vector.tensor_scalar(out=ri[:], in0=g[:], scalar1=127,
                                            scalar2=None,
                                            op0=mybir.AluOpType.bitwise_and)
                    nc.vector.tensor_scalar(
                        out=qi[:], in0=g[:], scalar1=7, scalar2=None,
                        op0=mybir.AluOpType.logical_shift_right)
                    rf = gpool.tile([P, cols], F32, tag="rf" + sfx)
                    qf = gpool.tile([P, cols], F32, tag="qf" + sfx)
                    nc.vector.tensor_copy(out=rf[:], in_=ri[:])
                    nc.vector.tensor_copy(out=qf[:], in_=qi[:])
                    return rf, qf

                rs_f, qs_f = rq(src_g, "s")
                rd_f, qd_f = rq(dst_g, "d")

                # gathers: (T, cp) pairs for src and dst; one column per call
                g_s = gpool.tile([P, cols, 2], F32, tag="g_s")
                g_d = gpool.tile([P, cols, 2], F32, tag="g_d")
                for c in range(cols):
                    nc.gpsimd.indirect_dma_start(
                        out=g_s[:, c, :], out_offset=None, in_=tab_d[:],
                        in_offset=bass.IndirectOffsetOnAxis(
                            ap=src_g[:, c:c + 1], axis=0))
                    nc.gpsimd.indirect_dma_start(
                        out=g_d[:, c, :], out_offset=None, in_=tab_d[:],
                        in_offset=bass.IndirectOffsetOnAxis(
                            ap=dst_g[:, c:c + 1], axis=0))

                t_s, cp_s = g_s[:, :, 0], g_s[:, :, 1]
                t_d, cp_d = g_d[:, :, 0], g_d[:, :, 1]

                def tt(a, b, op, tag, dt=F32):
                    o = gpool.tile([P, cols], dt, tag=tag)
                    nc.vector.tensor_tensor(out=o[:], in0=a, in1=b, op=op)
                    return o

                OP = mybir.AluOpType
                delta = tt(t_s, t_d, OP.subtract, "delta")
                nc.vector.tensor_scalar_max(out=delta[:], in0=delta[:],
                                            scalar1=0.0)
                rl = gpool.tile([P, cols], F32, tag="rl")
                nc.vector.reciprocal(out=rl[:], in_=l_g[:])
                grad = tt(delta[:], rl[:], OP.mult, "grad")
                gk = tt(grad[:], k_g[:], OP.mult, "gk")
                lng = gpool.tile([P, cols], F32, tag="lng")
                nc.scalar.activation(out=lng[:], in_=gk[:],
                                     func=mybir.ActivationFunctionType.Ln)
                hfd = gpool.tile([P, cols], F32, tag="hfd")
                nc.scalar.activation(out=hfd[:], in_=lng[:],
                                     func=mybir.ActivationFunctionType.Exp,
                                     scale=1.0 / 3.0)
                ec = tt(hfd[:], a_g[:], OP.mult, "ec")
                nc.vector.tensor_scalar(out=ec[:], in0=ec[:],
                                        scalar1=ts_b[:, 0:1], scalar2=None,
                                        op0=OP.mult)
                num = tt(cp_s, cp_d, OP.mult, "num")
                den = tt(cp_s, cp_d, OP.add, "den")
                nc.vector.reciprocal(out=den[:], in_=den[:])
                cpc = tt(num[:], den[:], OP.mult, "cpc")
                maxe = tt(delta[:], cpc[:], OP.mult, "maxe")
                e32 = tt(ec[:], maxe[:], OP.min, "e32")

                # scatter: per column one-hot matmuls into psum
                for c in range(cols):
                    hot_s = colpool.tile([P, P], F16, tag="hot_s")
                    nc.vector.tensor_scalar(
                        out=hot_s[:], in0=iota128[:],
                        scalar1=rs_f[:, c:c + 1], scalar2=e32[:, c:c + 1],
                        op0=OP.is_equal, op1=OP.mult)
                    qhot_s = colpool.tile([P, QDIM], F16, tag="qhot_s")
                    nc.vector.tensor_scalar(
                        out=qhot_s[:], in0=iotaq[:],
                        scalar1=qs_f[:, c:c + 1], scalar2=None,
                        op0=OP.is_equal)
                    hot_r = colpool.tile([P, P], F16, tag="hot_r")
                    nc.vector.tensor_scalar(
                        out=hot_r[:], in0=iota128[:],
                        scalar1=rd_f[:, c:c + 1], scalar2=e32[:, c:c + 1],
                        op0=OP.is_equal, op1=OP.mult)
                    qhot_r = colpool.tile([P, QDIM], F16, tag="qhot_r")
                    nc.vector.tensor_scalar(
                        out=qhot_r[:], in0=iotaq[:],
                        scalar1=qd_f[:, c:c + 1], scalar2=None,
                        op0=OP.is_equal)
                    first = c == 0
                    last = c == cols - 1
                    for lo, hi in ((0, 512), (512, QDIM)):
                        nc.tensor.matmul(out=psum_s[:, lo:hi], lhsT=hot_s[:],
                                         rhs=qhot_s[:, lo:hi], start=first,
                                         stop=last)
                        nc.tensor.matmul(out=psum_r[:, lo:hi], lhsT=hot_r[:],
                                         rhs=qhot_r[:, lo:hi], start=first,
                                         stop=last)

                # fold this tile's psum into the SBUF accumulators
                nc.vector.tensor_add(out=acc_s[:], in0=acc_s[:],
                                     in1=psum_s[:])
                nc.vector.tensor_add(out=acc_r[:], in0=acc_r[:],
                                     in1=psum_r[:])

            if iters > 1:
                with tc.For_i(0, iters, 1) as it:
                    body(it)
            else:
                body(0)

            nc.sync.dma_start(out=sent_d[:], in_=acc_s[:])
            nc.sync.dma_start(out=recv_d[:], in_=acc_r[:])

    nc.compile()
    return nc


def get_kernel(cols, iters, n_cores):
    key = (cols, iters, n_cores)
    if key not in _CACHE:
        _CACHE[key] = build_kernel(*key)
    return _CACHE[key]


def kernel(T, cp, L, conductivity, A, time_step, src, dst):
    T = np.asarray(T, np.float32)
    cp = np.asarray(cp, np.float32)
    L = np.asarray(L, np.float32)
    conductivity = np.asarray(conductivity, np.float32)
    A = np.asarray(A, np.float32)
    time_step = np.asarray(time_step, np.float32)
    src = np.asarray(src, np.int32)
    dst = np.asarray(dst, np.int32)

    n = T.shape[0]
    e = src.shape[0]

    cols = 128
    tile_e = P * cols
    epc_raw = (e + N_CORES - 1) // N_CORES
    iters = (epc_raw + tile_e - 1) // tile_e
    epc = iters * tile_e

    nc = get_kernel(cols, iters, N_CORES)

    tab = np.empty((N_NODES, 2), np.float32)
    tab[:n, 0] = T
    tab[:n, 1] = cp
    tab[n:] = 0.0

    in_maps = []
    for c in range(N_CORES):
        lo, hi = c * epc_raw, min((c + 1) * epc_raw, e)
        cnt = hi - lo

        def pad(x, fill, dt):
            out = np.full(epc, fill, dt)
            out[:cnt] = x[lo:hi]
            return out

        in_maps.append({
            "src": pad(src, 0, np.int32),
            "dst": pad(dst, 0, np.int32),
            "lw": pad(L, 1.0, np.float32),
            "kw": pad(conductivity, 0.0, np.float32),
            "aw": pad(A, 0.0, np.float32),
            "tab": tab,
            "ts": time_step.reshape(1, 1),
        })

    res = run_bass_kernel_spmd(nc, in_maps, core_ids=list(range(N_CORES)))

    acc = np.zeros((P, QDIM), np.float64)
    for c in range(N_CORES):
        acc += res.results[c]["recvp"].astype(np.float64)
        acc -= res.results[c]["sentp"].astype(np.float64)
    # node n = 128*q + r lives at [r, q]
    full = acc.T.ravel()[:n]
    return full.astype(np.float32)


# revision 13
# speedup vs baseline: 100.7473x; 1.1967x over previous
"""Trainium2 Bass kernel: GNN heat-conduction message passing.

Contract: kernel(**inputs) takes FULL numpy inputs (T, cp, L, conductivity,
A, time_step, src, dst) and returns the FULL [N] output
(heat_received - heat_sent), computed on 8 NeuronCores.

Strategy (edge-parallel, per sharding hint):
  - Shard the 6.4M edges across 8 cores (800k each, padded with zero-effect
    edges); replicate the small [N] node arrays T/cp to every core as an
    interleaved (T, cp) table in DRAM.
  - Per core: stream edge tiles of 128x128; gather (T, cp) pairs for src and
    dst endpoints via indirect DMA (embedding-gather pattern, 128 edges per
    call); per-edge math on DVE/ACT; segment-sums via per-column one-hot
    matmuls accumulated in two persistent PSUM banks laid out [r=128, q=784]
    where node n = 128*q + r.
  - Host unshard: reorder the [128, 784] partials to [N] and sum the 8 cores'
    (received - sent) partials.
"""

import numpy as np

import concourse.bacc as bacc
import concourse.bass as bass
import concourse.mybir as mybir
import concourse.tile as tile
from concourse.bass_utils import run_bass_kernel_spmd

N_NODES = 100_000
N_EDGES = 6_400_000
N_CORES = 8
P = 128
QDIM = 784  # node n = 128*q + r, q < ceil(100000/128)=782, padded to 784

F32 = mybir.dt.float32
F16 = mybir.dt.float16
I32 = mybir.dt.int32

_CACHE = {}


def build_kernel(cols, iters, n_cores):
    """Build the Bass module. Edge capacity per core = 128*cols*iters."""
    tile_e = P * cols
    epc = tile_e * iters

    nc = bacc.Bacc("TRN2", target_bir_lowering=False, debug=False,
                   enable_asserts=False, num_devices=n_cores)

    src_d = nc.dram_tensor("src", [epc], I32, kind="ExternalInput")
    dst_d = nc.dram_tensor("dst", [epc], I32, kind="ExternalInput")
    l_d = nc.dram_tensor("lw", [epc], F32, kind="ExternalInput")
    k_d = nc.dram_tensor("kw", [epc], F32, kind="ExternalInput")
    a_d = nc.dram_tensor("aw", [epc], F32, kind="ExternalInput")
    tab_d = nc.dram_tensor("tab", [N_NODES, 2], F32, kind="ExternalInput")
    ts_d = nc.dram_tensor("ts", [1, 1], F32, kind="ExternalInput")
    sent_d = nc.dram_tensor("sentp", [P, QDIM], F32, kind="ExternalOutput")
    recv_d = nc.dram_tensor("recvp", [P, QDIM], F32, kind="ExternalOutput")

    with tile.TileContext(nc) as tc:
        with (
            tc.tile_pool(name="const", bufs=1) as cpool,
            tc.tile_pool(name="grid", bufs=3) as gpool,
            tc.tile_pool(name="col", bufs=6) as colpool,
            tc.tile_pool(name="psum", bufs=1, space="PSUM") as ppool,
        ):
            # ---- constants ----
            iota128_i = cpool.tile([P, P], I32)
            nc.gpsimd.iota(iota128_i[:], pattern=[[1, P]], base=0,
                           channel_multiplier=0)
            iota128 = cpool.tile([P, P], F16)
            nc.vector.tensor_copy(out=iota128[:], in_=iota128_i[:])
            iotaq_i = cpool.tile([P, QDIM], I32)
            nc.gpsimd.iota(iotaq_i[:], pattern=[[1, QDIM]], base=0,
                           channel_multiplier=0)
            iotaq = cpool.tile([P, QDIM], F16)
            nc.vector.tensor_copy(out=iotaq[:], in_=iotaq_i[:])

            ts_sb = cpool.tile([1, 1], F32)
            nc.sync.dma_start(out=ts_sb[:], in_=ts_d[:])
            ts_b = cpool.tile([P, 1], F32)
            nc.gpsimd.partition_broadcast(ts_b[:], ts_sb[:])

            zeroq = cpool.tile([P, QDIM], F32)
            nc.vector.memset(zeroq[:], 0.0)
            zh = cpool.tile([P, P], F16)
            nc.vector.memset(zh[:], 0.0)
            zq = cpool.tile([P, QDIM], F16)
            nc.vector.memset(zq[:], 0.0)

            psum_s = ppool.tile([P, QDIM], F32)
            psum_r = ppool.tile([P, QDIM], F32)
            acc_s = cpool.tile([P, QDIM], F32)
            acc_r = cpool.tile([P, QDIM], F32)
            nc.vector.tensor_copy(out=acc_s[:], in_=zeroq[:])
            nc.vector.tensor_copy(out=acc_r[:], in_=zeroq[:])

            def body(it):
                base = it * tile_e

                def load(dram, dt, tag):
                    t = gpool.tile([P, cols], dt, tag=tag)
                    ap = dram[bass.ds(base, tile_e)].rearrange(
                        "(p c) -> p c", c=cols)
                    nc.sync.dma_start(out=t[:], in_=ap)
                    return t

                src_g = load(src_d, I32, "src_g")
                dst_g = load(dst_d, I32, "dst_g")
                l_g = load(l_d, F32, "l_g")
                k_g = load(k_d, F32, "k_g")
                a_g = load(a_d, F32, "a_g")

                # index fields -> fp16 grids
                def rq(g, sfx):
                    ri = gpool.tile([P, cols], I32, tag="ri" + sfx)
                    qi = gpool.tile([P, cols], I32, tag="qi" + sfx)
                    nc.vector.tensor_scalar(out=ri[:], in0=g[:], scalar1=127,
                                            scalar2=None,
                                            op0=mybir.AluOpType.bitwise_and)
                    nc.vector.tensor_scalar(
                        out=qi[:], in0=g[:], scalar1=7, scalar2=None,
                        op0=mybir.AluOpType.logical_shift_right)
                    rf = gpool.tile([P, cols], F32, tag="rf" + sfx)
                    qf = gpool.tile([P, cols], F32, tag="qf" + sfx)
                    nc.vector.tensor_copy(out=rf[:], in_=ri[:])
                    nc.vector.tensor_copy(out=qf[:], in_=qi[:])
                    return rf, qf

                rs_f, qs_f = rq(src_g, "s")
                rd_f, qd_f = rq(dst_g, "d")

                # gathers: (T, cp) pairs for src and dst; one column per call
                g_s = gpool.tile([P, cols, 2], F32, tag="g_s")
                g_d = gpool.tile([P, cols, 2], F32, tag="g_d")
                for c in range(cols):
                    nc.gpsimd.indirect_dma_start(
                        out=g_s[:, c, :], out_offset=None, in_=tab_d[:],
                        in_offset=bass.IndirectOffsetOnAxis(
                            ap=src_g[:, c:c + 1], axis=0))
                    nc.gpsimd.indirect_dma_start(
                        out=g_d[:, c, :], out_offset=None, in_=tab_d[:],
                        in_offset=bass.IndirectOffsetOnAxis(
                            ap=dst_g[:, c:c + 1], axis=0))

                t_s, cp_s = g_s[:, :, 0], g_s[:, :, 1]
                t_d, cp_d = g_d[:, :, 0], g_d[:, :, 1]

                def tt(a, b, op, tag, dt=F32):
                    o = gpool.tile([P, cols], dt, tag=tag)
                    nc.vector.tensor_tensor(out=o[:], in0=a, in1=b, op=op)
                    return o

                OP = mybir.AluOpType
                delta = tt(t_s, t_d, OP.subtract, "delta")
                nc.vector.tensor_scalar_max(out=delta[:], in0=delta[:],
                                            scalar1=0.0)
                rl = gpool.tile([P, cols], F32, tag="rl")
                nc.vector.reciprocal(out=rl[:], in_=l_g[:])
                grad = tt(delta[:], rl[:], OP.mult, "grad")
                gk = tt(grad[:], k_g[:], OP.mult, "gk")
                lng = gpool.tile([P, cols], F32, tag="lng")
                nc.scalar.activation(out=lng[:], in_=gk[:],
                                     func=mybir.ActivationFunctionType.Ln)
                hfd = gpool.tile([P, cols], F32, tag="hfd")
                nc.scalar.activation(out=hfd[:], in_=lng[:],
                                     func=mybir.ActivationFunctionType.Exp,
                                     scale=1.0 / 3.0)
                ec = tt(hfd[:], a_g[:], OP.mult, "ec")
                nc.vector.tensor_scalar(out=ec[:], in0=ec[:],
                                        scalar1=ts_b[:, 0:1], scalar2=None,
                                        op0=OP.mult)
                num = tt(cp_s, cp_d, OP.mult, "num")
                den = tt(cp_s, cp_d, OP.add, "den")
                nc.vector.reciprocal(out=den[:], in_=den[:])
                cpc = tt(num[:], den[:], OP.mult, "cpc")
                maxe = tt(delta[:], cpc[:], OP.mult, "maxe")
                e32 = tt(ec[:], maxe[:], OP.min, "e32")

                # scatter: per column one-hot matmuls into psum
                for c in range(cols):
                    hot_s = colpool.tile([P, P], F16, tag="hot_s")
                    nc.vector.tensor_scalar(
                        out=hot_s[:], in0=iota128[:],
                        scalar1=rs_f[:, c:c + 1], scalar2=e32[:, c:c + 1],
                        op0=OP.is_equal, op1=OP.mult)
                    qhot_s = colpool.tile([P, QDIM], F16, tag="qhot_s")
                    nc.vector.tensor_scalar(
                        out=qhot_s[:], in0=iotaq[:],
                        scalar1=qs_f[:, c:c + 1], scalar2=None,
                        op0=OP.is_equal)
                    hot_r = colpool.tile([P, P], F16, tag="hot_r")
                    nc.vector.tensor_scalar(
                        out=hot_r[:], in0=iota128[:],
                        scalar1=rd_f[:, c:c + 1], scalar2=e32[:, c:c + 1],
                        op0=OP.is_equal, op1=OP.mult)
                    qhot_r = colpool.tile([P, QDIM], F16, tag="qhot_r")
                    nc.vector.tensor_scalar(
                        out=qhot_r[:], in0=iotaq[:],
                        scalar1=qd_f[:, c:c + 1], scalar2=None,
                        op0=OP.is_equal)
                    first = c == 0
                    last = c == cols - 1
                    for lo, hi in ((0, 512), (512, QDIM)):
                        nc.tensor.matmul(out=psum_s[:, lo:hi], lhsT=hot_s[:],
                                         rhs=qhot_s[:, lo:hi], start=first,
                                         stop=last)
                        nc.tensor.matmul(out=psum_r[:, lo:hi], lhsT=hot_r[:],
                                         rhs=qhot_r[:, lo:hi], start=first,
                                         stop=last)

                # fold this tile's psum into the SBUF accumulators
                nc.vector.tensor_add(out=acc_s[:], in0=acc_s[:],
                                     in1=psum_s[:])
                nc.vector.tensor_add(out=acc_r[:], in0=acc_r[:],
                                     in1=psum_r[:])

            if iters > 1:
                with tc.For_i(0, iters, 1) as it:
                    body(it)
            else:
                body(0)

            nc.sync.dma_start(out=sent_d[:], in_=acc_s[:])
            nc.sync.dma_start(out=recv_d[:], in_=acc_r[:])

    nc.compile()
    return nc


def get_kernel(cols, iters, n_cores):
    key = (cols, iters, n_cores)
    if key not in _CACHE:
        _CACHE[key] = build_kernel(*key)
    return _CACHE[key]


def kernel(T, cp, L, conductivity, A, time_step, src, dst):
    T = np.asarray(T, np.float32)
    cp = np.asarray(cp, np.float32)
    L = np.asarray(L, np.float32)
    conductivity = np.asarray(conductivity, np.float32)
    A = np.asarray(A, np.float32)
    time_step = np.asarray(time_step, np.float32)
    src = np.asarray(src, np.int32)
    dst = np.asarray(dst, np.int32)

    n = T.shape[0]
    e = src.shape[0]

    cols = 128
    tile_e = P * cols
    epc_raw = (e + N_CORES - 1) // N_CORES
    iters = (epc_raw + tile_e - 1) // tile_e
    epc = iters * tile_e

    nc = get_kernel(cols, iters, N_CORES)

    tab = np.empty((N_NODES, 2), np.float32)
    tab[:n, 0] = T
    tab[:n, 1] = cp
    tab[n:] = 0.0

    in_maps = []
    for c in range(N_CORES):
        lo, hi = c * epc_raw, min((c + 1) * epc_raw, e)
        cnt = hi - lo

        def pad(x, fill, dt):
            out = np.full(epc, fill, dt)
            out[:cnt] = x[lo:hi]
            return out

        in_maps.append({
            "src": pad(src, 0, np.int32),
            "dst": pad(dst, 0, np.int32),
            "lw": pad(L, 1.0, np.float32),
            "kw": pad(conductivity, 0.0, np.float32),
            "aw": pad(A, 0.0, np.float32),
            "tab": tab,
            "ts": time_step.reshape(1, 1),
        })

    res = run_bass_kernel_spmd(nc, in_maps, core_ids=list(range(N_CORES)))

    acc = np.zeros((P, QDIM), np.float64)
    for c in range(N_CORES):
        acc += res.results[c]["recvp"].astype(np.float64)
        acc -= res.results[c]["sentp"].astype(np.float64)
    # node n = 128*q + r lives at [r, q]
    full = acc.T.ravel()[:n]
    return full.astype(np.float32)
